# revision 20
# baseline (speedup 1.0000x reference)
"""MoE (8 experts, top-2, cap-drop) Trainium2 kernel over 8 NeuronCores.

Strategy (expert-parallel, per sharding hint):
 - Router runs replicated on host (tiny: 134 MFLOP of the 344 GFLOP total) with
   the exact fp32 jax ops of the reference so top-2/capacity decisions match
   the oracle bit-for-bit; routing IS the sharding function here - it decides
   which token rows go to which expert shard.
 - Dispatch/shard: per expert e, gather its routed token rows (ascending token
   order, gate 0 on padding slots) and ship them transposed (D on partitions).
 - Each expert's FFN is split into 2 "units" along the hidden axis (2048 units
   each), giving 16 units; each core runs 2 units sequentially. Units are
   padded to a multiple of 128 token-columns and packed so the 8 largest go
   one-per-core (width mb*128) and the 8 smallest one-per-core (ma*128) -
   every core runs the identical (ma, mb) program shape (SPMD). Per unit:
       ysT = W2h^T-chain( relu( W1h^T-chain( xT ) ) )
   All matmuls in bf16 (fp32 PSUM accumulate): 1 PE cycle/row like f32r, but
   FWL weight loads, half the DMA bytes, and a small enough footprint that
   each unit's token tiles are SBUF-resident (one DMA per k-chunk per unit).
 - Combine/unshard: host sums the two half partial outputs per expert, applies
   the fp32 gates exactly as the reference (g[:,None] * h), and scatter-adds
   into y at the routed rows (row sets are disjoint within an expert).

Self-contained: shapes hardcoded for B=4, S=2048, D=1024, F=4096, E=8, top-2,
cap=2560, 8 cores.
"""

import sys

for _p in ("/opt/trn_rl_repo",):
    if _p not in sys.path:
        sys.path.append(_p)

import math
import os

import numpy as np

# "bf16" (default): bf16 matmuls, 1 PE cycle/row + FWL, ~3e-3 rel err.
# "tf32": f32r matmuls, 1 PE cycle/row, ~4e-4 rel err.
PRECISION = os.environ.get("KERNEL_PRECISION", "bf16")

B, S, D, F, E = 4, 2048, 1024, 4096, 8
TOP_K = 2
CAP_FACTOR = 1.25
T = B * S                                   # 8192 tokens
CAP = max(math.ceil(T * TOP_K * CAP_FACTOR / E), 1)   # 2560
FH = F // 2                                 # 2048 hidden units per core
NCORES = 8
BLK = 512                                   # token block (matmul moving dim)
P = 128


def _np_dtype():
    if PRECISION == "bf16":
        import ml_dtypes

        return np.dtype(ml_dtypes.bfloat16)
    return np.dtype(np.float32)


def _tf32_round(a: np.ndarray) -> np.ndarray:
    """Round-to-nearest-even fp32 -> tf32 (10-bit mantissa) on the host, so the
    on-device f32r matmuls see exactly-representable values."""
    u = a.reshape(-1).view(np.uint32).astype(np.uint64)
    r = (u + 0xFFF + ((u >> 13) & 1)) & ~np.uint64(0x1FFF)
    return r.astype(np.uint32).view(np.float32).reshape(a.shape)


def _to_dev(a: np.ndarray) -> np.ndarray:
    """Host-side cast of an fp32 array to the device matmul dtype."""
    if PRECISION == "bf16":
        return a.astype(_np_dtype())
    return _tf32_round(np.ascontiguousarray(a))


def _route(xf: np.ndarray, Wr: np.ndarray):
    """Replicate the reference's routing bit-for-bit on jax-CPU.

    Returns per-expert (idx[CAP] int64 token ids, gate[CAP] f32, 0 on padding).
    """
    import jax
    import jax.numpy as jnp

    cpu = jax.devices("cpu")[0]
    with jax.default_device(cpu):
        xj = jnp.asarray(xf, dtype=jnp.float32)
        wr = jnp.asarray(Wr, dtype=jnp.float32)
        probs = jax.nn.softmax(xj.astype(jnp.float32) @ wr, axis=-1)
        topk_probs, topk_experts = jax.lax.top_k(probs, TOP_K)
        idxs, gates = [], []
        for e in range(E):
            mask = topk_experts == e
            gate = jnp.sum(jnp.where(mask, topk_probs, 0.0), axis=-1)
            has = jnp.any(mask, axis=-1)
            g_masked = jnp.where(has, gate, -jnp.inf)
            vals, idx = jax.lax.top_k(g_masked, CAP)
            g = jnp.where(jnp.isfinite(vals), vals, 0.0)
            idxs.append(np.asarray(idx, dtype=np.int64))
            gates.append(np.asarray(g, dtype=np.float32))
    return idxs, gates


_COMPILED = {}


def _blocks_of(m: int):
    """Token-block widths for a unit m*128 columns wide: full 512-wide blocks
    plus one 128/256/384 tail (all still 1 PE cycle/column). The tail is
    placed SECOND so a unit always ends on a full block: the final block's
    compute hides the next unit's weight stream (and, for the last unit, the
    kernel never ends on short chains + sub-512B output DMA rows)."""
    full = [BLK] * (m // 4)
    if m % 4 == 0:
        return full
    tail = [(m % 4) * P]
    return full[:1] + tail + full[1:]


def _build(ma: int, mb: int):
    """Compile the SPMD per-core program: two sequential units of a dense
    relu-MLP half, ma / mb 128-col quarters wide respectively."""
    import concourse.bacc as bacc
    import concourse.mybir as mybir
    import concourse.tile as tile

    f32 = mybir.dt.float32
    mmdt = mybir.dt.bfloat16 if PRECISION == "bf16" else mybir.dt.float32r

    blocks = (_blocks_of(ma), _blocks_of(mb))
    nblk_a, nblk_b = len(blocks[0]), len(blocks[1])
    W = (ma * P, mb * P)                     # unit token widths

    nc = bacc.Bacc("TRN2", target_bir_lowering=False, debug=False,
                   num_devices=NCORES)
    KD = D // P      # 8  k-chunks for matmul 1
    KF = FH // P     # 16 k-chunks for matmul 2
    # per-unit token tiles, k-major: [k, p, j] - one contiguous DMA per chunk,
    # SBUF-resident for the whole unit
    xga = nc.dram_tensor("xga", [KD, P, W[0]], mmdt, kind="ExternalInput")
    xgb = nc.dram_tensor("xgb", [KD, P, W[1]], mmdt, kind="ExternalInput")
    xg_ts = (xga, xgb)
    # w1 host-pretiled f-major: [u, f, p, k*P+m] = W1h[k*P+p, f*P+m]
    w1 = nc.dram_tensor("w1", [2, KF, P, D], mmdt, kind="ExternalInput")
    # w2 host-pretiled d-major: [u, d, p, k2*P+m] = W2h[k2*P+p, d*P+m]
    w2 = nc.dram_tensor("w2", [2, KD, P, FH], mmdt, kind="ExternalInput")
    ysa = nc.dram_tensor("ysa", [KD, P, W[0]], mmdt, kind="ExternalOutput")
    ysb = nc.dram_tensor("ysb", [KD, P, W[1]], mmdt, kind="ExternalOutput")
    ys_ts = (ysa, ysb)
    warm = nc.dram_tensor("warm", [P, BLK // 2], f32, kind="ExternalOutput")

    with tile.TileContext(nc) as tc:
        with (
            tc.tile_pool(name="w1p", bufs=1) as w1p,
            tc.tile_pool(name="w2p", bufs=1) as w2p,
            tc.tile_pool(name="xgp", bufs=1) as xgp,
            tc.tile_pool(name="htp", bufs=1) as htp,
            tc.tile_pool(name="outp", bufs=2) as outp,
            tc.tile_pool(name="warmp", bufs=1) as warmp,
            tc.tile_pool(name="ps1", bufs=3, space="PSUM") as ps1,
            tc.tile_pool(name="ps2", bufs=3, space="PSUM") as ps2,
        ):
            # PE warm-up: dummy matmuls on a memset tile keep the HAM
            # activity monitor busy (full 2.4 GHz clock) while the first
            # real xg/W1 DMAs land; they depend on no DMA and start at t0.
            wsrc = warmp.tile([P, BLK // 2], mmdt, tag="warm_src")
            nc.vector.memset(wsrc[:], 0)
            wps = ps1.tile([P, BLK // 2], f32, tag="warm_ps", bufs=1)
            NWARM = 10
            for r in range(NWARM):
                nc.tensor.matmul(wps[:], wsrc[:, :P], wsrc[:],
                                 start=(r == 0), stop=(r == NWARM - 1))
            wout = warmp.tile([P, BLK // 2], f32, tag="warm_out")
            nc.vector.tensor_copy(wout[:], wps[:])
            nc.sync.dma_start(warm[:], wout[:])

            # Weight SBUF tiles are shared between the units (tagged by f/d
            # only): unit 1's load of tag f waits for unit 0's last read of
            # that tag, i.e. it streams during unit 0's final-block compute.
            w1sb = [None] * KF
            w2sb = [None] * KD

            def _load_w1(u, f):
                t = w1p.tile([P, D], mmdt, tag=f"w1_{f}")
                nc.sync.dma_start(t[:], w1[u, f])
                w1sb[f] = t

            def _load_w2(u, d):
                t = w2p.tile([P, FH], mmdt, tag=f"w2_{d}")
                nc.sync.dma_start(t[:], w2[u, d])
                w2sb[d] = t

            # Token-tile loads are split over BOTH HWDGE rings (sync +
            # scalar issue descriptors in parallel, ~650ns apiece) so the
            # 9-DMA critical set lands ~2x sooner. Only block 0's slice of
            # each chunk is loaded up front (subtile deps let block-0
            # matmuls start after ~1 MB); the rest streams during block 0.
            _load_w1(0, 0)
            xgu = [[None] * KD, [None] * KD]
            c0 = min(BLK, W[0])
            for k in range(KD):
                t = xgp.tile([P, W[0]], mmdt, tag=f"xga_{k}")
                eng = nc.scalar if k < KD // 2 else nc.sync
                eng.dma_start(t[:, :c0], xga[k][:, :c0])
                xgu[0][k] = t

            for u in range(2):
                off = 0
                for b, bw in enumerate(blocks[u]):
                    # during unit 0's final block, stream unit 1's weights in
                    # behind unit 0's last reads of each shared tile
                    last_a = u == 0 and b == nblk_a - 1
                    hts = []
                    for f in range(KF):
                        if u == 0 and b == 0 and f > 0:
                            # Stream the rest of unit 0's W1 at compute pace
                            # (f=0 already issued above).
                            _load_w1(0, f)
                        if (u == 0 and b == 0 and W[0] > BLK
                                and KD <= f < 2 * KD):
                            # Blocks 1+ of unit 0's token chunks stream in
                            # the second half of block 0's m1 (late enough
                            # not to crowd the W1 stream's bandwidth, early
                            # enough for block 2 at ~79us).
                            k = f - KD
                            nc.scalar.dma_start(xgu[0][k][:, c0:],
                                                xga[k][:, c0:])
                        if u == 0 and b == min(1, nblk_a - 1) and f == 1:
                            # Unit 1's token tiles have no dependencies;
                            # their ~4 MB stream during unit 0 compute.
                            for k in range(KD):
                                t = xgp.tile([P, W[1]], mmdt, tag=f"xgb_{k}")
                                nc.scalar.dma_start(t[:], xgb[k])
                                xgu[1][k] = t
                        ps = ps1.tile([P, BLK], f32)
                        for k in range(KD):
                            nc.tensor.matmul(
                                ps[:, :bw], w1sb[f][:, k * P:(k + 1) * P],
                                xgu[u][k][:, off:off + bw],
                                start=(k == 0), stop=(k == KD - 1))
                        ht = htp.tile([P, BLK], mmdt, tag=f"ht_{f}")
                        nc.scalar.activation(
                            ht[:, :bw], ps[:, :bw],
                            mybir.ActivationFunctionType.Relu)
                        hts.append(ht)
                        if last_a:
                            # Unit 1's W1 streams in behind unit 0's final
                            # read of each shared tile (emitted after the
                            # consuming chain so the chain sees unit 0 data).
                            _load_w1(1, f)
                        if u == 0 and b == 0 and f == KF - 1:
                            # Prefetch the first two W2 d-tiles at m1's end.
                            for dd in (0, 1):
                                _load_w2(0, dd)
                    for d in range(KD):
                        if u == 0 and b == 0 and d + 2 < KD:
                            # Stream the remaining d-tiles at m2 compute pace.
                            _load_w2(0, d + 2)
                        ps_ = ps2.tile([P, BLK], f32)
                        for k2 in range(KF):
                            nc.tensor.matmul(
                                ps_[:, :bw], w2sb[d][:, k2 * P:(k2 + 1) * P],
                                hts[k2][:, :bw],
                                start=(k2 == 0), stop=(k2 == KF - 1))
                        ob = outp.tile([P, BLK], mmdt)
                        nc.vector.tensor_copy(ob[:, :bw], ps_[:, :bw])
                        nc.sync.dma_start(ys_ts[u][d][:, off:off + bw],
                                          ob[:, :bw])
                        if last_a:
                            # As with W1: emitted after this d's consuming
                            # chain so unit 0's last block reads unit 0 data.
                            _load_w2(1, d)
                    off += bw
    nc.compile()
    return nc


def _get_compiled(ma: int, mb: int):
    key = (PRECISION, ma, mb)
    if key not in _COMPILED:
        _COMPILED[key] = _build(ma, mb)
    return _COMPILED[key]


def kernel(x, Wr, W1, W2, _timing=None):
    from concourse.bass_utils import run_bass_kernel_spmd

    x = np.asarray(x, dtype=np.float32)
    Wr = np.asarray(Wr, dtype=np.float32)
    W1 = np.asarray(W1, dtype=np.float32)
    W2 = np.asarray(W2, dtype=np.float32)
    xf = x.reshape(T, D)

    # --- Host router (replicated, reference-exact) => sharding plan ---
    idxs, gates = _route(xf, Wr)
    counts = [int(np.count_nonzero(gates[e])) for e in range(E)]
    # unit width in 128-col quarters
    msz = [max(1, math.ceil(c / P)) for c in counts]

    # --- Pack the 16 (expert, half) units onto 8 cores, 2 units each.
    # Sorted by size, the 8 largest units go one-per-core at width mb*128 and
    # the 8 smallest one-per-core at ma*128: identical SPMD shape everywhere,
    # total padded width 8*(m_1st + m_9th)*128 columns.
    units = sorted(((e, h) for e in range(E) for h in range(2)),
                   key=lambda u: -msz[u[0]])
    mb = msz[units[0][0]]
    ma = msz[units[8][0]]
    # Run the tail-free unit (width divisible by 512) last when possible, so
    # the kernel ends on a full-width block (short tail chains + sub-512B
    # output DMA rows would otherwise sit on the critical path).
    if ma % 4 == 0 or mb % 4 != 0:
        m0, m1 = mb, ma
        assign = [(units[c], units[8 + c]) for c in range(NCORES)]
    else:
        m0, m1 = ma, mb
        assign = [(units[8 + c], units[c]) for c in range(NCORES)]
    if os.environ.get("KERNEL_DEBUG"):
        print(f"[kernel] counts={counts} msz={msz} m0={m0} m1={m1}")

    # --- Dispatch: gather + transpose routed rows per expert ---
    xfT = np.ascontiguousarray(xf.T)                       # [D, T]
    W_ = (m0 * P, m1 * P)
    in_maps = []
    for c in range(NCORES):
        parts_w1, parts_w2 = [], []
        xg_parts = []
        for s, (e, h) in enumerate(assign[c]):
            # k-major token tile: [k, p, j]; padded cols gather garbage rows
            # (idxs pads past count) that the host gates to zero on combine
            xge = xfT[:, idxs[e][:W_[s]]]                  # [D, W]
            xg_parts.append(np.ascontiguousarray(
                xge.reshape(D // P, P, W_[s])))
            # f-major pretile: [f, p, k*P+m] = W1h[k*P+p, f*P+m]
            w1h = W1[e][:, h * FH:(h + 1) * FH]
            w1t = w1h.reshape(D // P, P, FH // P, P).transpose(2, 1, 0, 3)
            parts_w1.append(w1t.reshape(FH // P, P, D))
            # d-major pretile: [d, p, k2*P+m] = W2h[k2*P+p, d*P+m]
            w2h = W2[e][h * FH:(h + 1) * FH, :]
            w2t = w2h.reshape(FH // P, P, D // P, P).transpose(2, 1, 0, 3)
            parts_w2.append(w2t.reshape(D // P, P, FH))
        in_maps.append({
            "xga": _to_dev(xg_parts[0]),
            "xgb": _to_dev(xg_parts[1]),
            "w1": _to_dev(np.ascontiguousarray(np.stack(parts_w1))),
            "w2": _to_dev(np.ascontiguousarray(np.stack(parts_w2))),
        })

    # --- Device: 16 FFN half-units on 8 cores ---
    nc = _get_compiled(m0, m1)
    res = run_bass_kernel_spmd(
        nc, in_maps, list(range(NCORES)),
        trace=(_timing is not None),
        trace_cores=list(range(NCORES)) if _timing is not None else None,
    )
    if _timing is not None:
        _timing["exec_time_ns"] = res.exec_time_ns
        _timing["results"] = res

    # --- Combine/unshard: sum halves per expert, gate, scatter-add (host) ---
    part = {}                                  # (e, h) -> [D, W]
    for c in range(NCORES):
        for s, (e, h) in enumerate(assign[c]):
            ys = res.results[c]["ysa" if s == 0 else "ysb"]   # [k, p, j]
            part[(e, h)] = np.asarray(ys, dtype=np.float32).reshape(D, -1)
    y = np.zeros((T, D), dtype=np.float32)
    for e in range(E):
        p0, p1 = part[(e, 0)], part[(e, 1)]
        n = min(p0.shape[1], p1.shape[1])
        hs = p0[:, :n] + p1[:, :n]             # [D, n]
        y[idxs[e][:n]] += gates[e][:n, None] * hs.T
    return y.reshape(B, S, D)
